# revision 59
# baseline (speedup 1.0000x reference)
"""Gated-attention (Qwen-style) Trainium2 kernel, v2 (bf16).

Sharding (8 cores): data-parallel over batch (2) x tensor-parallel over head
groups (4). Core c handles batch b=c//4 and head group g=c%4: q heads
4g..4g+3, kv heads 2g..2g+1, gate logits 4g..4g+3, w_o columns 512g..512g+512.
Each core computes a partial output y_g = attn_out_g @ w_o[:, cols_g].T in
bf16; the host sums the 4 partials per batch in fp32.

v2 changes vs v1:
- bf16 operands everywhere on the PE (fp32 PSUM accumulation): enables fast
  weight load (halves LDWEIGHTS), halves SBUF footprint and HBM traffic.
- V projected directly to [tokens, d] layout with x-chunk-stationary matmuls
  (no PE transposes, no extra evictions).
- exp batched over two PSUM banks per ACT instruction ([128,1024]) and double
  buffered so ACT hides fully behind PE.
- softmax denominator: DVE add-tree folds the 16 exp k-tiles to one
  [128, 512] partial, gpsimd partition_all_reduce sums the partition axis
  (result replicated on all partitions), reciprocal_approx_fast inverts it.
  No denominator matmuls, no PSUM bank for it, and the scale is available
  per-head so the PSUM->SBUF eviction fuses with the gate/denominator
  scaling in a single scalar_tensor_tensor op.
- sigmoid(gate) rows are flattened + partition-broadcast once per kernel.
- x/weights loaded with full-row 1MB-class DMAs up front; y written as
  [128, 2048] bf16 tiles (host upcasts and reduces).
"""

import os
from contextlib import ExitStack

import numpy as np

B, S, HID = 2, 2048, 2048
NH, NKV, HD = 16, 8, 128
GATE = NH
KV_DIM = NKV * HD

N_CORES = 8
TPG = 4            # tensor-parallel group size (head groups)
QH = NH // TPG     # q heads per core = 4
KVH = NKV // TPG   # kv heads per core = 2
IB = 512           # phase-1 token block
NB = S // IB       # 4 blocks
JT = S // 128      # 16 key tiles
IBLK = 512         # phase-2 query block
NI = S // IBLK     # 4 query blocks
QKVC = 8 * HD + 4  # 1028 qkv+gate columns per core (q 512, k 256, gate 4, v 256)
SCALE = 1.0 / float(np.sqrt(HD))

_CACHE = {}

LAST_EXEC_NS = None
LAST_RESULTS = None


def _build_program():
    import concourse.bass as bass
    import concourse.bass_isa as bass_isa
    import concourse.mybir as mybir
    from concourse import bacc
    from concourse.tile import TileContext

    F32 = mybir.dt.float32
    BF16 = mybir.dt.bfloat16
    AF = mybir.ActivationFunctionType
    _REDUCE_ADD = bass_isa.ReduceOp.add
    _ALU_MULT = mybir.AluOpType.mult

    nc = bacc.Bacc()

    xT_d = nc.dram_tensor("xT", [HID, S], BF16, kind="ExternalInput")
    wqkvT_d = nc.dram_tensor("wqkvT", [HID, QKVC], BF16, kind="ExternalInput")
    woT_d = nc.dram_tensor("woT", [QH * HD, HID], BF16, kind="ExternalInput")
    cosT_d = nc.dram_tensor("cosT", [HD, S], BF16, kind="ExternalInput")
    sinT_d = nc.dram_tensor("sinT", [HD, S], BF16, kind="ExternalInput")
    rotm_d = nc.dram_tensor("rotm", [HD, HD], BF16, kind="ExternalInput")
    y_d = nc.dram_tensor("y", [S, HID], BF16, kind="ExternalOutput")

    with TileContext(nc) as tc, ExitStack() as persist:
        const = persist.enter_context(tc.tile_pool(name="const", bufs=1))
        rotm_sb = const.tile([HD, HD], BF16, tag="rotm", name="rotm_sb")

        qk_pool = persist.enter_context(tc.tile_pool(name="qk", bufs=1))
        qk_sb = [qk_pool.tile([128, S], BF16, tag=f"qk{r}", name=f"qk{r}")
                 for r in range(QH + KVH)]
        v_pool = persist.enter_context(tc.tile_pool(name="v", bufs=1))
        v_sb = [v_pool.tile([128, KVH * HD], BF16, tag=f"v{t}", name=f"v{t}")
                for t in range(JT)]
        g_pool = persist.enter_context(tc.tile_pool(name="g", bufs=1))
        sg4 = g_pool.tile([QH, S], BF16, tag="sg4", name="sg4")
        sgflat = g_pool.tile([1, QH * S], BF16, tag="sgflat", name="sgflat")
        sgBC = [g_pool.tile([128, S], BF16, tag=f"sgBC{h}", name=f"sgBC{h}")
                for h in range(QH)]
        # wo allocated outside the phase pools so its SBUF space never aliases
        # phase-1 tiles (else its DMA waits for phase 1's last reader); the
        # DMAs themselves are emitted after the x/w loads below.
        wopool = persist.enter_context(tc.tile_pool(name="wo", bufs=1))
        wo_sb = [wopool.tile([128, HID], BF16, tag=f"wo{i}", name=f"wo{i}")
                 for i in range(QH)]

        # ---------------- phase 1: qkv projection + rope + direct-v ---------
        with ExitStack() as ph1:
            wpool = ph1.enter_context(tc.tile_pool(name="w", bufs=1))
            wsb = [wpool.tile([128, QKVC], BF16, tag=f"w{h}", name=f"w{h}")
                   for h in range(16)]
            xpool = ph1.enter_context(tc.tile_pool(name="x", bufs=1))
            xb = [xpool.tile([128, S], BF16, tag=f"x{h}", name=f"x{h}")
                  for h in range(16)]
            for h in range(16):
                # block 0's inputs first so the first matmuls start sooner
                nc.sync.dma_start(out=wsb[h], in_=wqkvT_d[128 * h:128 * (h + 1), :])
                nc.sync.dma_start(out=xb[h][:, 0:IB],
                                  in_=xT_d[128 * h:128 * (h + 1), 0:IB])
            for h in range(16):
                nc.sync.dma_start(out=xb[h][:, IB:S],
                                  in_=xT_d[128 * h:128 * (h + 1), IB:S])
            # consts, cos/sin, wo after x/w: none needed until later
            nc.sync.dma_start(out=rotm_sb, in_=rotm_d[:, :])
            for cc in range(QH):
                nc.sync.dma_start(out=wo_sb[cc],
                                  in_=woT_d[128 * cc:128 * (cc + 1), :])
            cspool = ph1.enter_context(tc.tile_pool(name="cs", bufs=1))
            cs_sb = cspool.tile([HD, S], BF16, tag="cs", name="cs_sb")
            nc.sync.dma_start(out=cs_sb, in_=cosT_d[:, :])
            sn_sb = cspool.tile([HD, S], BF16, tag="sn", name="sn_sb")
            nc.sync.dma_start(out=sn_sb, in_=sinT_d[:, :])

            gpool = ph1.enter_context(tc.tile_pool(name="gf", bufs=1))
            gf32 = gpool.tile([QH, S], F32, tag="gf32", name="gf32")
            tmppool = ph1.enter_context(tc.tile_pool(name="tmp", bufs=3))

            ps_acc = ph1.enter_context(tc.tile_pool(name="acc", bufs=3, space="PSUM"))
            ps_rot = ph1.enter_context(tc.tile_pool(name="rot", bufs=2, space="PSUM"))
            ps_v = ph1.enter_context(tc.tile_pool(name="psv", bufs=2, space="PSUM"))
            ps_g = ph1.enter_context(tc.tile_pool(name="psg", bufs=1, space="PSUM"))

            for ib in range(NB):
                sl = slice(IB * ib, IB * (ib + 1))
                # gate logits [4, IB] first, so the last block's sigmoid chain
                # (DVE-heavy [4,2048] ops) runs while PE does its row-tiles
                # instead of sitting at the phase-1 DVE tail blocking the
                # phase-2 PSUM bank reuse
                psg = ps_g.tile([QH, IB], F32, tag="psg", name="psg")
                for h in range(16):
                    nc.tensor.matmul(psg, wsb[h][:, 6 * HD:6 * HD + 4],
                                     xb[h][:, sl], start=(h == 0), stop=(h == 15))
                nc.vector.tensor_copy(gf32[:, sl], psg)
                if ib == NB - 1:
                    # sigmoid(gate) natively on ACT (idle in phase 1; the two
                    # table-set switches hide in phase-1 slack)
                    nc.scalar.activation(out=sg4, in_=gf32, func=AF.Sigmoid)
                    # flatten + broadcast while gpsimd is idle in phase 1, so
                    # the gpsimd queue is empty when phase 2's first QK syncs
                    for h4 in range(QH):
                        nc.sync.dma_start(out=sgflat[0:1, S * h4:S * (h4 + 1)],
                                          in_=sg4[h4:h4 + 1, :])
                    for h4 in range(QH):
                        nc.gpsimd.partition_broadcast(
                            sgBC[h4], sgflat[0:1, S * h4:S * (h4 + 1)],
                            channels=128)
                # q/k row-tiles with rope
                for r in range(QH + KVH):
                    acc = ps_acc.tile([128, IB], F32, tag="acc", name="acc")
                    for h in range(16):
                        nc.tensor.matmul(acc, wsb[h][:, 128 * r:128 * (r + 1)],
                                         xb[h][:, sl], start=(h == 0), stop=(h == 15))
                    craw = tmppool.tile([128, IB], BF16, tag="craw", name="craw")
                    nc.vector.tensor_copy(craw, acc)
                    rps = ps_rot.tile([128, IB], F32, tag="rot", name="rot")
                    nc.tensor.matmul(rps, rotm_sb, craw, start=True, stop=True)
                    t1 = tmppool.tile([128, IB], BF16, tag="t1", name="t1")
                    nc.vector.tensor_mul(t1, craw, cs_sb[:, sl])
                    t2 = tmppool.tile([128, IB], BF16, tag="t2", name="t2")
                    nc.vector.tensor_mul(t2, rps, sn_sb[:, sl])
                    nc.vector.tensor_add(qk_sb[r][:, sl], t1, t2)
                # v directly in [tokens, d]: x-chunk stationary, wv moving
                for t2i in range(IB // 128):
                    tt = (IB // 128) * ib + t2i
                    vps = ps_v.tile([128, KVH * HD], F32, tag="vps", name="vps")
                    for h in range(16):
                        nc.tensor.matmul(
                            vps, xb[h][:, 128 * tt:128 * (tt + 1)],
                            wsb[h][:, 6 * HD + 4:QKVC],
                            start=(h == 0), stop=(h == 15))
                    nc.vector.tensor_copy(v_sb[tt], vps)

        # ---------------- phase 2: attention + gate + out-projection --------
        with ExitStack() as ph2:
            oc_pool = ph2.enter_context(tc.tile_pool(name="oc", bufs=2))
            epool = ph2.enter_context(tc.tile_pool(name="e", bufs=3))
            tr0pool = ph2.enter_context(tc.tile_pool(name="tr0", bufs=10))
            trpool = ph2.enter_context(tc.tile_pool(name="tr", bufs=4))
            denpool = ph2.enter_context(tc.tile_pool(name="den", bufs=8))
            rdpool = ph2.enter_context(tc.tile_pool(name="rd", bufs=4))
            ypool = ph2.enter_context(tc.tile_pool(name="y", bufs=4))

            ps_s = ph2.enter_context(tc.tile_pool(name="pss", bufs=2, space="PSUM"))
            ps_o = ph2.enter_context(tc.tile_pool(name="pso", bufs=2, space="PSUM"))
            ps_y = ph2.enter_context(tc.tile_pool(name="psy", bufs=2, space="PSUM"))

            for i in range(NI):
                isl = slice(IBLK * i, IBLK * (i + 1))
                oc_i = []
                rden_i = []
                for h in range(QH):
                    kv = h // 2
                    if h == 1 and i > 0:
                        # normalize block i-1's outputs now: the all_reduce
                        # chain is long done, and out-proj later won't stall
                        for h0 in range(QH):
                            rden = rdpool.tile([128, IBLK], F32, tag="rden",
                                               name="rden")
                            nc.vector.reciprocal_approx_fast(
                                out=rden, in_=rden_prev[h0])
                            nc.vector.tensor_mul(oc_prev[h0], oc_prev[h0], rden)
                    pso = ps_o.tile([128, IBLK], F32, tag="pso", name="pso")
                    lvl = []
                    for jp in range(JT // 2):
                        ps2 = ps_s.tile([128, 1024], F32, tag="pss", name="pss")
                        for j2 in range(2):
                            j = 2 * jp + j2
                            nc.tensor.matmul(
                                ps2[:, 512 * j2:512 * (j2 + 1)],
                                qk_sb[QH + kv][:, 128 * j:128 * (j + 1)],
                                qk_sb[h][:, isl], start=True, stop=True)
                        e2 = epool.tile([128, 1024], BF16, tag="e2", name="e2")
                        nc.scalar.activation(out=e2, in_=ps2, func=AF.Exp, scale=SCALE)
                        for j2 in range(2):
                            j = 2 * jp + j2
                            first = (jp == 0 and j2 == 0)
                            last = (jp == JT // 2 - 1 and j2 == 1)
                            nc.tensor.matmul(
                                pso, v_sb[j][:, 128 * kv:128 * (kv + 1)],
                                e2[:, 512 * j2:512 * (j2 + 1)],
                                start=first, stop=last)
                        # denominator partial: fold the two 512-wide k-tiles
                        a0 = tr0pool.tile([128, IBLK], BF16, tag="tr0", name="tr0")
                        nc.vector.tensor_add(a0, e2[:, 0:512], e2[:, 512:1024])
                        lvl.append(a0)
                    # add-tree 8 -> 4 -> 2 -> 1 k-partials on DVE
                    li = 1
                    while len(lvl) > 1:
                        nxt = []
                        for m in range(0, len(lvl), 2):
                            b0 = trpool.tile([128, IBLK], BF16, tag=f"tr{li}",
                                             name=f"tr{li}")
                            nc.vector.tensor_add(b0, lvl[m], lvl[m + 1])
                            nxt.append(b0)
                        lvl = nxt
                        li += 1
                    # denominator summed across partitions, replicated to all;
                    # this chain is OFF the PE critical path: pso is evicted
                    # with the gate-only scale, 1/denom is applied one block
                    # later when rden is long ready.
                    den_bc = denpool.tile([128, IBLK], F32, tag="den", name="den")
                    nc.gpsimd.partition_all_reduce(den_bc, lvl[0], channels=128,
                                                   reduce_op=_REDUCE_ADD)
                    rden_i.append(den_bc)
                    # fused eviction: oc = pso * sigmoid(gate)  (no den dep)
                    oc = oc_pool.tile([128, IBLK], BF16, tag=f"oc{h}", name=f"oc{h}")
                    nc.vector.scalar_tensor_tensor(
                        out=oc, in0=pso, scalar=1.0, in1=sgBC[h][:, isl],
                        op0=_ALU_MULT, op1=_ALU_MULT)
                    oc_i.append(oc)

                # out-projection pipelined one block behind (oc already
                # normalized mid-attention above)
                def out_proj(i0, oc_blk):
                    for t2i in range(IBLK // 128):
                        t = (IBLK // 128) * i0 + t2i
                        ysb = ypool.tile([128, HID], BF16, tag="ysb", name="ysb")
                        for o in range(4):
                            psy = ps_y.tile([128, IBLK], F32, tag="psy", name="psy")
                            for cc in range(QH):
                                nc.tensor.matmul(
                                    psy, oc_blk[cc][:, 128 * t2i:128 * (t2i + 1)],
                                    wo_sb[cc][:, IBLK * o:IBLK * (o + 1)],
                                    start=(cc == 0), stop=(cc == QH - 1))
                            nc.scalar.copy(ysb[:, IBLK * o:IBLK * (o + 1)], psy)
                        nc.sync.dma_start(out=y_d[128 * t:128 * (t + 1), :], in_=ysb)

                if i > 0:
                    out_proj(i - 1, oc_prev)
                oc_prev, rden_prev = oc_i, rden_i
            # final block: normalize then project
            for h0 in range(QH):
                rden = rdpool.tile([128, IBLK], F32, tag="rden", name="rden")
                nc.vector.reciprocal_approx_fast(out=rden, in_=rden_prev[h0])
                nc.vector.tensor_mul(oc_prev[h0], oc_prev[h0], rden)
            out_proj(NI - 1, oc_prev)

    nc.finalize()
    return nc


def kernel(hidden_states, cos, sin, w_qkv, w_o):
    global LAST_EXEC_NS, LAST_RESULTS
    import ml_dtypes
    from concourse.bass_utils import run_bass_kernel_spmd

    BF = ml_dtypes.bfloat16
    hidden_states = np.asarray(hidden_states, dtype=np.float32)
    w_qkv = np.asarray(w_qkv, dtype=np.float32)
    w_o = np.asarray(w_o, dtype=np.float32)

    if "nc" not in _CACHE:
        _CACHE["nc"] = _build_program()
    nc = _CACHE["nc"]

    cosT = np.ascontiguousarray(np.asarray(cos, dtype=np.float32).T).astype(BF)
    sinT = np.ascontiguousarray(np.asarray(sin, dtype=np.float32).T).astype(BF)
    rotm = np.zeros((HD, HD), dtype=np.float32)
    for i in range(HD // 2):
        rotm[i + HD // 2, i] = -1.0   # rot[d'] = -q[d'+64] for d' < 64
        rotm[i, i + HD // 2] = 1.0    # rot[d'] = +q[d'-64] for d' >= 64
    rotm = rotm.astype(BF)

    xT = [np.ascontiguousarray(hidden_states[b].T).astype(BF) for b in range(B)]
    in_maps = []
    for c in range(N_CORES):
        b, g = divmod(c, TPG)
        qr = w_qkv[512 * g:512 * (g + 1)]
        kr = w_qkv[HID + GATE + 256 * g:HID + GATE + 256 * (g + 1)]
        gr = w_qkv[HID + QH * g:HID + QH * (g + 1)]
        vr = w_qkv[HID + GATE + KV_DIM + 256 * g:HID + GATE + KV_DIM + 256 * (g + 1)]
        wqkvT = np.ascontiguousarray(
            np.concatenate([qr, kr, gr, vr], axis=0).T).astype(BF)
        woT = np.ascontiguousarray(w_o[:, 512 * g:512 * (g + 1)].T).astype(BF)
        in_maps.append({
            "xT": xT[b], "wqkvT": wqkvT, "woT": woT,
            "cosT": cosT, "sinT": sinT, "rotm": rotm,
        })

    trace = bool(int(os.environ.get("KERNEL_TRACE", "0")))
    out = run_bass_kernel_spmd(nc, in_maps, list(range(N_CORES)), trace=trace)
    LAST_EXEC_NS = out.exec_time_ns
    LAST_RESULTS = out
    y = np.zeros((B, S, HID), dtype=np.float32)
    for c in range(N_CORES):
        b = c // TPG
        y[b] += out.results[c]["y"].astype(np.float32)
    return y
